# revision 3
# baseline (speedup 1.0000x reference)
"""Self-contained 2-layer GCN kernel for 8 Trainium2 NeuronCores.

kernel(**inputs) takes the FULL unsharded inputs (x, edge_index, W1, b1,
W2, b2) and returns the full [N, 128] float32 output.

Design:
- Target-node blocks (128 nodes) are load-balanced across (core, slot)
  pairs so all 8 cores run one identical SPMD program; per-core work is
  equalized via a shared per-slot chunk-count template (shortfall padded
  with dummy edges whose one-hot column is -1 -> zero contribution).
- Layer 1 uses (A_hat X) W1 associativity: edges gather rows of the
  host-prepared dinv*x table (bf16) with dma_gather, a 0/1 one-hot
  [slot,target] matrix is built on VectorE (is_equal vs iota) and the
  128-edge chunk is accumulated into PSUM via TensorE: psum += oh.T @ msg.
- Per-slot epilogue: transpose agg (PE), dense W1 (+relu, dinv scale on
  ScalarE), transpose h, dense W2 -> xws2 (bf16).
- xws2 is AllGathered into the layer-2 table in 4 slot-quarter
  sub-collectives so the exchange overlaps remaining layer-1 compute;
  layer-2 repeats the aggregation at F=128 and writes f32 output shards.
- dma_gather indices are int16, so tables are split in lo/hi halves of
  25088 rows; the host groups each slot's edges per half.
"""
import numpy as np
import ml_dtypes

import jax
from jax.sharding import Mesh, PartitionSpec
from jax.experimental.shard_map import shard_map

import concourse.bacc as bacc
import concourse.mybir as mybir
import concourse.tile as tile
from concourse.bass2jax import _bass_exec_p, install_neuronx_cc_hook, partition_id_tensor

P = 128
F32 = mybir.dt.float32
BF16 = mybir.dt.bfloat16
I16 = mybir.dt.int16
NP_BF16 = ml_dtypes.bfloat16


# ----------------------------------------------------------------------------
# Host-side planning
# ----------------------------------------------------------------------------

def _pack_idx(vals: np.ndarray) -> np.ndarray:
    """Pack an int16 index stream into the [128, n/16] dma_gather layout.

    Position i is read from idxs[i % 16, i // 16]; the 16-row pattern is
    replicated 8x down the partitions (one copy per Q7 core).
    """
    n = len(vals)
    assert n % 16 == 0
    arr16 = np.asarray(vals, np.int16).reshape(n // 16, 16).T  # [16, n/16]
    return np.tile(arr16, (8, 1))  # [128, n/16]


class LayerPlan:
    """Per-layer gather/one-hot plan: per-core idx streams + csel + template."""

    def __init__(self, srcs, tgt_core, tgt_slot, tgt_off, n_cores, slots, half):
        # group edges by (core, slot, half-of-source)
        e_half = (srcs >= half).astype(np.int64)
        e_idx = np.where(e_half == 0, srcs, srcs - half).astype(np.int64)
        assert e_idx.max() < 2 ** 15
        key = ((tgt_core * slots + tgt_slot) * 2 + e_half)
        order = np.argsort(key, kind="stable")
        key_s = key[order]
        idx_s = e_idx[order]
        off_s = tgt_off[order]
        n_groups = n_cores * slots * 2
        counts = np.bincount(key_s, minlength=n_groups).reshape(n_cores, slots, 2)
        chunks = -(-counts // P)  # ceil div
        # template: per (slot, half) chunk count = max over cores
        self.K = chunks.max(axis=0)  # [slots, 2]
        starts = np.concatenate([[0], np.cumsum(counts.reshape(-1))])
        self.idx_streams = []   # per core: (lo_vals, hi_vals)
        self.csel = []          # per core: [128, n_chunks] float (-1 pad)
        nch = int(self.K.sum())
        self.n_chunks = nch
        for c in range(n_cores):
            lo_parts, hi_parts = [], []
            cs = np.full((nch, P), -1.0, np.float32)
            ck = 0
            for j in range(slots):
                for h in range(2):
                    g = (c * slots + j) * 2 + h
                    cnt = counts[c, j, h]
                    kk = int(self.K[j, h])
                    vals = np.zeros(kk * P, np.int64)
                    sel = np.full(kk * P, -1.0, np.float32)
                    vals[:cnt] = idx_s[starts[g]:starts[g] + cnt]
                    sel[:cnt] = off_s[starts[g]:starts[g] + cnt]
                    (lo_parts if h == 0 else hi_parts).append(vals)
                    cs[ck:ck + kk] = sel.reshape(kk, P)
                    ck += kk
            lo = np.concatenate(lo_parts) if lo_parts else np.zeros(0, np.int64)
            hi = np.concatenate(hi_parts) if hi_parts else np.zeros(0, np.int64)
            self.idx_streams.append((lo, hi))
            self.csel.append(cs.T.copy())  # [128, n_chunks]
        self.tot = (int(self.K[:, 0].sum()) * P, int(self.K[:, 1].sum()) * P)


def plan_host(x, edge_index, W1, b1, W2, b2, n_cores=8):
    N, F1 = x.shape
    F2 = W2.shape[1]
    row = np.asarray(edge_index[0], np.int64)
    col = np.asarray(edge_index[1], np.int64)

    nb = -(-N // P)
    nbp = -(-nb // n_cores) * n_cores          # padded #blocks (392)
    slots = nbp // n_cores                     # 49
    npad = nbp * P                             # 50176
    half = npad // 2                           # 25088
    assert half % P == 0 and half < 2 ** 15

    deg = np.bincount(col, minlength=N).astype(np.float64) + 1.0
    dinv = (deg ** -0.5).astype(np.float32)

    # all edges incl. self loops
    loops = np.arange(N, dtype=np.int64)
    srcs = np.concatenate([row, loops])
    tgts = np.concatenate([col, loops])

    # --- balance target blocks across (core, slot) ---
    blk = tgts // P
    eb = np.bincount(blk, minlength=nbp)
    order = np.argsort(-eb, kind="stable")     # blocks by load desc
    assign = order.reshape(slots, n_cores)     # assign[j, c] = block id
    core_of_blk = np.empty(nbp, np.int64)
    slot_of_blk = np.empty(nbp, np.int64)
    new_base = np.empty(nbp, np.int64)
    # layer-2 table layout is quarter-major: [(quarter, core, slot-in-q), 128]
    # so the AllGather can run as 4 contiguous sub-collectives overlapped
    # with layer-1 compute.
    NQ = 4
    qsize = [slots // NQ + (1 if q < slots % NQ else 0) for q in range(NQ)]
    qslot0 = np.concatenate([[0], np.cumsum(qsize)])[:NQ]
    quarter_of_slot = np.repeat(np.arange(NQ), qsize)
    for j in range(slots):
        q = quarter_of_slot[j]
        for c in range(n_cores):
            b = assign[j, c]
            core_of_blk[b] = c
            slot_of_blk[b] = j
            new_base[b] = (int(qslot0[q]) * n_cores + c * qsize[q]
                           + (j - int(qslot0[q]))) * P
    new_row = new_base[np.arange(npad) // P] + np.arange(npad) % P  # node -> table2 row

    tgt_core = core_of_blk[blk]
    tgt_slot = slot_of_blk[blk]
    tgt_off = (tgts % P).astype(np.float32)

    l1 = LayerPlan(srcs, tgt_core, tgt_slot, tgt_off, n_cores, slots, half)
    l2 = LayerPlan(new_row[srcs], tgt_core, tgt_slot, tgt_off, n_cores, slots, half)

    # --- tables / constants ---
    xs = np.zeros((npad, F1), NP_BF16)
    xs[:N] = (x.astype(np.float32) * dinv[:, None]).astype(NP_BF16)

    dinv_pad = np.zeros(npad, np.float32)
    dinv_pad[:N] = dinv
    iota = np.tile(np.arange(P, dtype=np.float32), (P, 1)).astype(NP_BF16)
    ident = np.eye(P, dtype=np.float32).astype(NP_BF16)

    in_maps = []
    for c in range(n_cores):
        m = {
            "xs": xs,
            "w1": W1.astype(np.float32).astype(NP_BF16),
            "w2": W2.astype(np.float32).astype(NP_BF16),
            "b1r": np.tile(np.asarray(b1, np.float32), (P, 1)),
            "b2r": np.tile(np.asarray(b2, np.float32), (P, 1)),
            "iota": iota,
            "ident": ident,
            # dinv of this core's blocks, [128, slots] (partition = within-block)
            "dtgt": dinv_pad[assign[:, c][:, None] * P
                             + np.arange(P)[None, :]].T.copy(),
            "cs1": l1.csel[c].astype(NP_BF16),
            "cs2": l2.csel[c].astype(NP_BF16),
            "idx1l": _pack_idx(l1.idx_streams[c][0]),
            "idx1h": _pack_idx(l1.idx_streams[c][1]),
            "idx2l": _pack_idx(l2.idx_streams[c][0]),
            "idx2h": _pack_idx(l2.idx_streams[c][1]),
        }
        in_maps.append(m)

    meta = {
        "N": N, "F1": F1, "F2": F2, "n_cores": n_cores,
        "b1_zero": bool(np.all(np.asarray(b1) == 0)),
        "b2_zero": bool(np.all(np.asarray(b2) == 0)),
        "slots": slots, "npad": npad, "half": half,
        "K1": l1.K, "K2": l2.K,
        "tot1": l1.tot, "tot2": l2.tot,
        "nch1": l1.n_chunks, "nch2": l2.n_chunks,
        "assign": assign,
        "qsize": qsize, "qslot0": [int(v) for v in qslot0],
    }
    return in_maps, meta


def assemble_output(shards, meta):
    """shards: list per core of [slots*128, F2] -> full [N, F2]."""
    n_cores, slots = meta["n_cores"], meta["slots"]
    F2, N, npad = meta["F2"], meta["N"], meta["npad"]
    assign = meta["assign"]
    out = np.empty((npad, F2), shards[0].dtype)
    for j in range(slots):
        for c in range(n_cores):
            b = assign[j, c]
            out[b * P:(b + 1) * P] = shards[c][j * P:(j + 1) * P]
    return out[:N]


# ----------------------------------------------------------------------------
# Device program
# ----------------------------------------------------------------------------

class GatherStream:
    """Issues batched dma_gathers for one (table-half, layer) idx stream and
    hands out per-chunk rhs APs."""

    def __init__(self, nc, pool, table_ap, idx_tile, total_idx, feat, tag,
                 slab_chunks=32, bufs=2):
        self.nc = nc
        self.pool = pool
        self.table_ap = table_ap
        self.idx_tile = idx_tile
        self.total = total_idx
        self.feat = feat
        self.tag = tag
        self.slab = slab_chunks
        self.bufs = bufs
        self.pos = 0              # chunk cursor
        self.cur_tile = None

    def next_chunk(self):
        s, c = divmod(self.pos, self.slab)
        if c == 0:
            base = s * self.slab * P
            n_idx = min(self.slab * P, self.total - base)
            k = n_idx // P
            t = self.pool.tile([P, self.slab, self.feat], BF16, tag=self.tag,
                               bufs=self.bufs)
            self.nc.gpsimd.dma_gather(
                out_ap=t[:, :k, :],
                in_ap=self.table_ap,
                idxs_ap=self.idx_tile[:, base // 16:(base + n_idx) // 16],
                num_idxs=n_idx,
                num_idxs_reg=n_idx,
                elem_size=self.feat,
            )
            self.cur_tile = t
        self.pos += 1
        return self.cur_tile[:, c, :]


def build_nc(meta, slab_chunks=32, n_cores=None, collective=True, io_only=False,
             oh_batch=8, dma_scratch=65536):
    n_cores = n_cores or meta["n_cores"]
    slots, npad, half = meta["slots"], meta["npad"], meta["half"]
    F1, F2 = meta["F1"], meta["F2"]
    K1, K2 = meta["K1"], meta["K2"]
    nch1, nch2 = meta["nch1"], meta["nch2"]
    nsh = slots * P

    nc = bacc.Bacc(num_devices=n_cores, dynamic_dma_scratch_size=dma_scratch)
    dp = nc.declare_dram_parameter
    xs = dp("xs", [npad, F1], BF16, isOutput=False)
    w1 = dp("w1", [F1, F1], BF16, isOutput=False)
    w2 = dp("w2", [F1, F2], BF16, isOutput=False)
    b1r = dp("b1r", [P, F1], F32, isOutput=False)
    b2r = dp("b2r", [P, F2], F32, isOutput=False)
    iota = dp("iota", [P, P], BF16, isOutput=False)
    ident = dp("ident", [P, P], BF16, isOutput=False)
    dtgt = dp("dtgt", [P, slots], F32, isOutput=False)
    cs1 = dp("cs1", [P, nch1], BF16, isOutput=False)
    cs2 = dp("cs2", [P, nch2], BF16, isOutput=False)
    idx1l = dp("idx1l", [P, meta["tot1"][0] // 16], I16, isOutput=False)
    idx1h = dp("idx1h", [P, meta["tot1"][1] // 16], I16, isOutput=False)
    idx2l = dp("idx2l", [P, meta["tot2"][0] // 16], I16, isOutput=False)
    idx2h = dp("idx2h", [P, meta["tot2"][1] // 16], I16, isOutput=False)
    tick = dp("tick", [1, 4], F32, isOutput=False)
    out = dp("out", [nsh, F2], F32, isOutput=True)
    tock = dp("tock", [1, 4], F32, isOutput=True)

    qsize = meta.get("qsize", [slots])
    qslot0 = meta.get("qslot0", [0])
    NQ = len(qsize)
    xws2q = [nc.dram_tensor(f"xws2q{q}", [qsize[q] * P, F2], BF16)
             for q in range(NQ)]
    tab2 = nc.dram_tensor("tab2", [npad, F2], BF16, addr_space="Shared")
    q_of_slot = []
    for q in range(NQ):
        q_of_slot += [q] * qsize[q]

    AL = mybir.AluOpType
    ACT = mybir.ActivationFunctionType

    with tile.TileContext(nc) as tc:
        # NOTE: Bacc.compile() auto-inserts the GPSIMD library load for
        # dma_gather (insert_library_loads pass) -- no manual load_library.
        with (
            tc.tile_pool(name="const", bufs=1) as cpool,
            tc.tile_pool(name="msg", bufs=2) as mpool,
            tc.tile_pool(name="work", bufs=2) as wpool,
            tc.tile_pool(name="psum", bufs=2, space="PSUM") as ppool,
        ):
            # timing passthrough: tock = tick (chained-repeat measurement)
            tick_t = cpool.tile([1, 4], F32, tag="tick", bufs=1)
            nc.sync.dma_start(tick_t[:], tick[:, :])
            nc.sync.dma_start(tock[:, :], tick_t[:])

            def load_const(ap, shape, dtype, name):
                t = cpool.tile(shape, dtype, tag=name, bufs=1)
                nc.sync.dma_start(t[:], ap)
                return t

            w1_t = cpool.tile([P, 2, F1], BF16, tag="w1", bufs=1)
            for k in range(2):
                nc.sync.dma_start(w1_t[:, k, :], w1[k * P:(k + 1) * P, :])
            w2_t = cpool.tile([P, 2, F2], BF16, tag="w2", bufs=1)
            for k in range(2):
                nc.sync.dma_start(w2_t[:, k, :], w2[k * P:(k + 1) * P, :])
            b1_t = load_const(b1r[:, :], [P, F1], F32, "b1")
            b2_t = load_const(b2r[:, :], [P, F2], F32, "b2")
            io_t = load_const(iota[:, :], [P, P], BF16, "iota")
            id_t = load_const(ident[:, :], [P, P], BF16, "ident")
            dt_t = load_const(dtgt[:, :], [P, slots], F32, "dtgt")
            cs1_t = load_const(cs1[:, :], [P, nch1], BF16, "cs1")
            cs2_t = load_const(cs2[:, :], [P, nch2], BF16, "cs2")
            i1l_t = load_const(idx1l[:, :], [P, meta["tot1"][0] // 16], I16, "i1l")
            i1h_t = load_const(idx1h[:, :], [P, meta["tot1"][1] // 16], I16, "i1h")
            i2l_t = load_const(idx2l[:, :], [P, meta["tot2"][0] // 16], I16, "i2l")
            i2h_t = load_const(idx2h[:, :], [P, meta["tot2"][1] // 16], I16, "i2h")

            if io_only:
                zt = wpool.tile([P, F2], F32, tag="ep2")
                nc.vector.memset(zt[:], 0.0)
                for j in range(slots):
                    nc.sync.dma_start(out[j * P:(j + 1) * P, :], zt[:])

            hT = cpool.tile([P, 2, nsh], BF16, tag="hT", bufs=1)

            st1 = [
                GatherStream(nc, mpool, xs[0:half, :], i1l_t, meta["tot1"][0], F1, "m1l",
                             slab_chunks),
                GatherStream(nc, mpool, xs[half:npad, :], i1h_t, meta["tot1"][1], F1, "m1h",
                             slab_chunks),
            ]
            st2 = [
                GatherStream(nc, mpool, tab2[0:half, :], i2l_t, meta["tot2"][0], F2, "m2l",
                             slab_chunks),
                GatherStream(nc, mpool, tab2[half:npad, :], i2h_t, meta["tot2"][1], F2, "m2h",
                             slab_chunks),
            ]

            def aggregate(j, K, streams, cs_t, ck0, feat):
                """One-hot matmul accumulation for slot j; returns psum tile."""
                psum = ppool.tile([P, feat], F32, tag="agg")
                nch = int(K[j, 0] + K[j, 1])
                ci = 0
                ck = ck0
                for h in range(2):
                    left = int(K[j, h])
                    while left > 0:
                        nb = min(oh_batch, left)
                        # build nb one-hots in a single DVE op
                        oh = wpool.tile([P, oh_batch, P], BF16, tag="oh", bufs=3)
                        nc.vector.tensor_tensor(
                            out=oh[:, :nb, :],
                            in0=cs_t[:, ck:ck + nb, None].to_broadcast([P, nb, P]),
                            in1=io_t[:, None, :].to_broadcast([P, nb, P]),
                            op=AL.is_equal,
                        )
                        for i in range(nb):
                            msg = streams[h].next_chunk()
                            nc.tensor.matmul(psum[:], lhsT=oh[:, i, :], rhs=msg,
                                             start=(ci == 0), stop=(ci == nch - 1))
                            ci += 1
                        ck += nb
                        left -= nb
                return psum, ck

            ck1 = 0
            for j in range(slots if not io_only else 0):
                # ---- layer-1 aggregation over raw x_s ----
                psum, ck1 = aggregate(j, K1, st1, cs1_t, ck1, F1)
                aggc = wpool.tile([P, F1], BF16, tag="aggc")
                nc.scalar.activation(aggc[:], psum[:], ACT.Copy)
                aggT = wpool.tile([P, 2, P], BF16, tag="aggT")
                for k in range(2):
                    ptr = ppool.tile([P, P], BF16, tag="tr")
                    nc.tensor.transpose(ptr[:], aggc[:, k * P:(k + 1) * P], id_t[:])
                    nc.vector.tensor_copy(aggT[:, k, :], ptr[:])
                # ---- dense W1 + epilogue ----
                pd = ppool.tile([P, F1], F32, tag="dense")
                for k in range(2):
                    nc.tensor.matmul(pd[:], lhsT=aggT[:, k, :], rhs=w1_t[:, k, :],
                                     start=(k == 0), stop=(k == 1))
                htile = wpool.tile([P, F1], BF16, tag="h")
                if meta.get("b1_zero", False):
                    # h = relu(dinv * (agg@W1)) in one ACT pass from PSUM
                    nc.scalar.activation(htile[:], pd[:], ACT.Relu,
                                         scale=dt_t[:, j:j + 1])
                else:
                    t1 = wpool.tile([P, F1], F32, tag="ep1")
                    nc.vector.tensor_scalar(t1[:], pd[:], dt_t[:, j:j + 1], None,
                                            op0=AL.mult)
                    nc.vector.tensor_tensor(t1[:], t1[:], b1_t[:], op=AL.add)
                    nc.scalar.activation(htile[:], t1[:], ACT.Relu)
                for k in range(2):
                    ptr = ppool.tile([P, P], BF16, tag="tr")
                    nc.tensor.transpose(ptr[:], htile[:, k * P:(k + 1) * P], id_t[:])
                    nc.vector.tensor_copy(hT[:, k, j * P:(j + 1) * P], ptr[:])
                # ---- dense W2 -> xws2 ----
                pd2 = ppool.tile([P, F2], F32, tag="dense")
                for k in range(2):
                    nc.tensor.matmul(pd2[:], lhsT=hT[:, k, j * P:(j + 1) * P],
                                     rhs=w2_t[:, k, :], start=(k == 0), stop=(k == 1))
                xw2t = wpool.tile([P, F2], BF16, tag="xw2")
                nc.scalar.activation(xw2t[:], pd2[:], ACT.Copy,
                                     scale=dt_t[:, j:j + 1])
                q = q_of_slot[j]
                jq = j - qslot0[q]
                nc.sync.dma_start(xws2q[q][jq * P:(jq + 1) * P, :], xw2t[:])
                if jq == qsize[q] - 1:
                    # last slot of this quarter: fire its sub-AllGather so it
                    # overlaps with the remaining layer-1 slots
                    r0 = qslot0[q] * n_cores * P
                    r1 = r0 + qsize[q] * n_cores * P
                    if collective:
                        nc.gpsimd.collective_compute(
                            "AllGather",
                            AL.bypass,
                            replica_groups=[list(range(n_cores))],
                            ins=[xws2q[q].ap().opt()],
                            outs=[tab2[r0:r1, :].opt()],
                        )
                    else:
                        nc.sync.dma_start(
                            tab2[r0:r0 + qsize[q] * P, :], xws2q[q][:, :])

            # ---- layer-2 aggregation + epilogue ----
            ck2 = 0
            for j in range(slots if not io_only else 0):
                psum, ck2 = aggregate(j, K2, st2, cs2_t, ck2, F2)
                t2 = wpool.tile([P, F2], F32, tag="ep2")
                if meta.get("b2_zero", False):
                    nc.scalar.activation(t2[:], psum[:], ACT.Copy,
                                         scale=dt_t[:, j:j + 1])
                else:
                    nc.vector.tensor_scalar(t2[:], psum[:], dt_t[:, j:j + 1], None,
                                            op0=AL.mult)
                    nc.vector.tensor_tensor(t2[:], t2[:], b2_t[:], op=AL.add)
                nc.sync.dma_start(out[j * P:(j + 1) * P, :], t2[:])

    nc.compile()
    return nc


class SpmdRunner:
    def __init__(self, nc, n_cores: int = 8, nreps: int = 1,
                 tick_name: str = "tick", tock_name: str = "tock"):
        install_neuronx_cc_hook()
        self.nc = nc
        self.n_cores = n_cores
        assert nc.dbg_addr is None or not nc.dbg_callbacks
        self.dbg_name = nc.dbg_addr.name if nc.dbg_addr is not None else None
        partition_name = nc.partition_id_tensor.name if nc.partition_id_tensor else None

        in_names, out_names, out_avals = [], [], []
        zero_outs = []
        for alloc in nc.m.functions[0].allocations:
            if not isinstance(alloc, mybir.MemoryLocationSet):
                continue
            name = alloc.memorylocations[0].name
            if alloc.kind == "ExternalInput":
                if name != partition_name:
                    in_names.append(name)
            elif alloc.kind == "ExternalOutput":
                out_names.append(name)
                shape = tuple(alloc.tensor_shape)
                dtype = mybir.dt.np(alloc.dtype)
                out_avals.append(jax.core.ShapedArray(shape, dtype))
                zero_outs.append(np.zeros(shape, dtype))
        self.in_names = in_names      # order matters; includes dbg if declared
        self.out_names = out_names
        self.out_avals = out_avals
        self.zero_outs = zero_outs
        n_params = len(in_names)
        n_outs = len(out_avals)
        all_in_names = list(in_names) + list(out_names)
        if partition_name is not None:
            all_in_names.append(partition_name)

        tick_i = in_names.index(tick_name) if (nreps > 1 and tick_name in in_names) else None
        tock_i = out_names.index(tock_name) if (nreps > 1 and tock_name in out_names) else None
        assert nreps == 1 or (tick_i is not None and tock_i is not None), \
            "nreps>1 needs tick/tock passthrough tensors in the kernel"

        def _call(operands):
            if partition_name is not None:
                operands = operands + [partition_id_tensor()]
            return _bass_exec_p.bind(
                *operands,
                out_avals=tuple(out_avals),
                in_names=tuple(all_in_names),
                out_names=tuple(out_names),
                lowering_input_output_aliases=(),
                sim_require_finite=True,
                sim_require_nnan=True,
                nc=nc,
            )

        def _body(*args):
            operands = list(args)
            outs = _call(list(operands))
            for _ in range(nreps - 1):
                operands2 = list(operands)
                operands2[tick_i] = outs[tock_i]
                outs = _call(operands2)
            return tuple(outs)

        devices = jax.devices()[: self.n_cores]
        assert len(devices) == self.n_cores
        mesh = Mesh(np.asarray(devices), ("core",))
        in_specs = (PartitionSpec("core"),) * (n_params + n_outs)
        out_specs = (PartitionSpec("core"),) * n_outs
        # NOTE: no donation so we can reuse the zero buffers across timed calls.
        self._fn = jax.jit(
            shard_map(_body, mesh=mesh, in_specs=in_specs, out_specs=out_specs,
                      check_rep=False),
            keep_unused=True,
        )
        self._concat_zeros = [
            np.zeros((self.n_cores * z.shape[0], *z.shape[1:]), z.dtype)
            for z in zero_outs
        ]
        self._dev_zeros = None
        self._dev_in = None

    def stage_inputs(self, in_maps):
        """in_maps: list (len n_cores) of dict name->np.ndarray."""
        if self.dbg_name is not None:
            in_maps = [
                {**m, self.dbg_name: np.zeros((1, 2), np.uint32)} for m in in_maps
            ]
        concat_in = [
            np.concatenate([np.asarray(in_maps[c][name]) for c in range(self.n_cores)],
                           axis=0)
            for name in self.in_names
        ]
        self._dev_in = [jax.device_put(a) for a in concat_in]
        self._dev_zeros = [jax.device_put(a) for a in self._concat_zeros]
        jax.block_until_ready(self._dev_in)
        jax.block_until_ready(self._dev_zeros)

    def run(self):
        outs = self._fn(*self._dev_in, *self._dev_zeros)
        jax.block_until_ready(outs)
        return outs

    def run_chain(self, n):
        """Dispatch n executions back-to-back (tick chained through tock to
        force strict ordering), block once at the end."""
        ti = self.in_names.index("tick")
        oi = self.out_names.index("tock")
        ins = list(self._dev_in)
        outs = self._fn(*ins, *self._dev_zeros)
        for _ in range(n - 1):
            ins[ti] = outs[oi]
            outs = self._fn(*ins, *self._dev_zeros)
        jax.block_until_ready(outs)
        return outs

    def results(self, outs):
        return [
            {
                name: np.asarray(outs[i]).reshape(self.n_cores, *self.out_avals[i].shape)[c]
                for i, name in enumerate(self.out_names)
            }
            for c in range(self.n_cores)
        ]


# ----------------------------------------------------------------------------
# Public entry point
# ----------------------------------------------------------------------------

_CACHE = {}


def kernel(**inputs) -> np.ndarray:
    x = np.asarray(inputs["x"], np.float32)
    edge_index = np.asarray(inputs["edge_index"], np.int64)
    W1 = np.asarray(inputs["W1"], np.float32)
    b1 = np.asarray(inputs["b1"], np.float32)
    W2 = np.asarray(inputs["W2"], np.float32)
    b2 = np.asarray(inputs["b2"], np.float32)

    in_maps, meta = plan_host(x, edge_index, W1, b1, W2, b2)
    for m in in_maps:
        m["tick"] = np.zeros((1, 4), np.float32)

    key = (x.shape, edge_index.shape, W2.shape,
           tuple(meta["K1"].reshape(-1)), tuple(meta["K2"].reshape(-1)),
           meta["b1_zero"], meta["b2_zero"])
    if key not in _CACHE:
        nc = build_nc(meta, slab_chunks=8, oh_batch=8)
        _CACHE[key] = SpmdRunner(nc, meta["n_cores"])
    runner = _CACHE[key]
    runner.stage_inputs(in_maps)
    outs = runner.run()
    res = runner.results(outs)
    shards = [res[c]["out"] for c in range(meta["n_cores"])]
    return assemble_output(shards, meta).astype(np.float32)


# revision 4
# speedup vs baseline: 6.3235x; 6.3235x over previous
"""Self-contained 2-layer GCN kernel for 8 Trainium2 NeuronCores.

kernel(**inputs) takes the FULL unsharded inputs (x, edge_index, W1, b1,
W2, b2) and returns the full [N, 128] float32 output.

Design:
- Target-node blocks (128 nodes) are load-balanced across (core, slot)
  pairs so all 8 cores run one identical SPMD program; per-core work is
  equalized via a shared per-slot chunk-count template (shortfall padded
  with dummy edges whose one-hot column is -1 -> zero contribution).
- Layer 1 uses (A_hat X) W1 associativity: edges gather rows of the
  host-prepared dinv*x table (bf16) with dma_gather, a 0/1 one-hot
  [slot,target] matrix is built on VectorE (is_equal vs iota, 16 chunks
  per op) and each 128-edge chunk accumulates into PSUM on TensorE:
  psum += oh.T @ msg.
- Per-slot epilogue: transpose agg (PE), dense W1 (+relu, dinv scale on
  ScalarE), transpose h, dense W2 -> xws2 (bf16).
- xws2 is AllGathered into the layer-2 table in 4 slot-quarter
  sub-collectives so the exchange overlaps remaining layer-1 compute;
  layer-2 repeats the aggregation at F=128 and writes f32 output shards.
- dma_gather indices are int16, so tables are split in lo/hi halves of
  25088 rows; gathers run as 24-chunk (3072-index) slabs with
  single_packet=False (larger slabs hang the SWDGE path otherwise).
"""
import numpy as np
import ml_dtypes

import jax
from jax.sharding import Mesh, PartitionSpec
from jax.experimental.shard_map import shard_map

import concourse.bacc as bacc
import concourse.mybir as mybir
import concourse.tile as tile
from concourse.bass2jax import _bass_exec_p, install_neuronx_cc_hook, partition_id_tensor

P = 128
F32 = mybir.dt.float32
BF16 = mybir.dt.bfloat16
I16 = mybir.dt.int16
NP_BF16 = ml_dtypes.bfloat16


# ----------------------------------------------------------------------------
# Host-side planning
# ----------------------------------------------------------------------------

def _pack_idx(vals: np.ndarray) -> np.ndarray:
    """Pack an int16 index stream into the [128, n/16] dma_gather layout.

    Position i is read from idxs[i % 16, i // 16]; the 16-row pattern is
    replicated 8x down the partitions (one copy per Q7 core).
    """
    n = len(vals)
    assert n % 16 == 0
    arr16 = np.asarray(vals, np.int16).reshape(n // 16, 16).T  # [16, n/16]
    return np.tile(arr16, (8, 1))  # [128, n/16]


class LayerPlan:
    """Per-layer gather/one-hot plan: per-core idx streams + csel + template."""

    def __init__(self, srcs, tgt_core, tgt_slot, tgt_off, n_cores, slots, half):
        # group edges by (core, slot, half-of-source)
        e_half = (srcs >= half).astype(np.int64)
        e_idx = np.where(e_half == 0, srcs, srcs - half).astype(np.int64)
        assert e_idx.max() < 2 ** 15
        key = ((tgt_core * slots + tgt_slot) * 2 + e_half)
        order = np.argsort(key, kind="stable")
        key_s = key[order]
        idx_s = e_idx[order]
        off_s = tgt_off[order]
        n_groups = n_cores * slots * 2
        counts = np.bincount(key_s, minlength=n_groups).reshape(n_cores, slots, 2)
        chunks = -(-counts // P)  # ceil div
        # template: per (slot, half) chunk count = max over cores
        self.K = chunks.max(axis=0)  # [slots, 2]
        starts = np.concatenate([[0], np.cumsum(counts.reshape(-1))])
        self.idx_streams = []   # per core: (lo_vals, hi_vals)
        self.csel = []          # per core: [128, n_chunks] float (-1 pad)
        nch = int(self.K.sum())
        self.n_chunks = nch
        for c in range(n_cores):
            lo_parts, hi_parts = [], []
            cs = np.full((nch, P), -1.0, np.float32)
            ck = 0
            for j in range(slots):
                for h in range(2):
                    g = (c * slots + j) * 2 + h
                    cnt = counts[c, j, h]
                    kk = int(self.K[j, h])
                    vals = np.zeros(kk * P, np.int64)
                    sel = np.full(kk * P, -1.0, np.float32)
                    vals[:cnt] = idx_s[starts[g]:starts[g] + cnt]
                    sel[:cnt] = off_s[starts[g]:starts[g] + cnt]
                    (lo_parts if h == 0 else hi_parts).append(vals)
                    cs[ck:ck + kk] = sel.reshape(kk, P)
                    ck += kk
            lo = np.concatenate(lo_parts) if lo_parts else np.zeros(0, np.int64)
            hi = np.concatenate(hi_parts) if hi_parts else np.zeros(0, np.int64)
            self.idx_streams.append((lo, hi))
            self.csel.append(cs.T.copy())  # [128, n_chunks]
        self.tot = (int(self.K[:, 0].sum()) * P, int(self.K[:, 1].sum()) * P)


def plan_host(x, edge_index, W1, b1, W2, b2, n_cores=8):
    N, F1 = x.shape
    F2 = W2.shape[1]
    row = np.asarray(edge_index[0], np.int64)
    col = np.asarray(edge_index[1], np.int64)

    nb = -(-N // P)
    nbp = -(-nb // n_cores) * n_cores          # padded #blocks (392)
    slots = nbp // n_cores                     # 49
    npad = nbp * P                             # 50176
    half = npad // 2                           # 25088
    assert half % P == 0 and half < 2 ** 15

    deg = np.bincount(col, minlength=N).astype(np.float64) + 1.0
    dinv = (deg ** -0.5).astype(np.float32)

    # all edges incl. self loops
    loops = np.arange(N, dtype=np.int64)
    srcs = np.concatenate([row, loops])
    tgts = np.concatenate([col, loops])

    # --- balance target blocks across (core, slot) ---
    blk = tgts // P
    eb = np.bincount(blk, minlength=nbp)
    order = np.argsort(-eb, kind="stable")     # blocks by load desc
    assign = order.reshape(slots, n_cores)     # assign[j, c] = block id
    core_of_blk = np.empty(nbp, np.int64)
    slot_of_blk = np.empty(nbp, np.int64)
    new_base = np.empty(nbp, np.int64)
    # layer-2 table layout is quarter-major: [(quarter, core, slot-in-q), 128]
    # so the AllGather can run as 4 contiguous sub-collectives overlapped
    # with layer-1 compute.
    NQ = 4
    qsize = [slots // NQ + (1 if q < slots % NQ else 0) for q in range(NQ)]
    qslot0 = np.concatenate([[0], np.cumsum(qsize)])[:NQ]
    quarter_of_slot = np.repeat(np.arange(NQ), qsize)
    for j in range(slots):
        q = quarter_of_slot[j]
        for c in range(n_cores):
            b = assign[j, c]
            core_of_blk[b] = c
            slot_of_blk[b] = j
            new_base[b] = (int(qslot0[q]) * n_cores + c * qsize[q]
                           + (j - int(qslot0[q]))) * P
    new_row = new_base[np.arange(npad) // P] + np.arange(npad) % P  # node -> table2 row

    tgt_core = core_of_blk[blk]
    tgt_slot = slot_of_blk[blk]
    tgt_off = (tgts % P).astype(np.float32)

    l1 = LayerPlan(srcs, tgt_core, tgt_slot, tgt_off, n_cores, slots, half)
    l2 = LayerPlan(new_row[srcs], tgt_core, tgt_slot, tgt_off, n_cores, slots, half)

    # --- tables / constants ---
    xs = np.zeros((npad, F1), NP_BF16)
    xs[:N] = (x.astype(np.float32) * dinv[:, None]).astype(NP_BF16)

    dinv_pad = np.zeros(npad, np.float32)
    dinv_pad[:N] = dinv
    iota = np.tile(np.arange(P, dtype=np.float32), (P, 16)).astype(NP_BF16)
    ident = np.eye(P, dtype=np.float32).astype(NP_BF16)

    in_maps = []
    for c in range(n_cores):
        m = {
            "xs": xs,
            "w1": W1.astype(np.float32).astype(NP_BF16),
            "w2": W2.astype(np.float32).astype(NP_BF16),
            "b1r": np.tile(np.asarray(b1, np.float32), (P, 1)),
            "b2r": np.tile(np.asarray(b2, np.float32), (P, 1)),
            "iota": iota,
            "ident": ident,
            # dinv of this core's blocks, [128, slots] (partition = within-block)
            "dtgt": dinv_pad[assign[:, c][:, None] * P
                             + np.arange(P)[None, :]].T.copy(),
            "cs1": l1.csel[c].astype(NP_BF16),
            "cs2": l2.csel[c].astype(NP_BF16),
            "idx1l": _pack_idx(l1.idx_streams[c][0]),
            "idx1h": _pack_idx(l1.idx_streams[c][1]),
            "idx2l": _pack_idx(l2.idx_streams[c][0]),
            "idx2h": _pack_idx(l2.idx_streams[c][1]),
        }
        in_maps.append(m)

    meta = {
        "N": N, "F1": F1, "F2": F2, "n_cores": n_cores,
        "b1_zero": bool(np.all(np.asarray(b1) == 0)),
        "b2_zero": bool(np.all(np.asarray(b2) == 0)),
        "slots": slots, "npad": npad, "half": half,
        "K1": l1.K, "K2": l2.K,
        "tot1": l1.tot, "tot2": l2.tot,
        "nch1": l1.n_chunks, "nch2": l2.n_chunks,
        "assign": assign,
        "qsize": qsize, "qslot0": [int(v) for v in qslot0],
    }
    return in_maps, meta


def assemble_output(shards, meta):
    """shards: list per core of [slots*128, F2] -> full [N, F2]."""
    n_cores, slots = meta["n_cores"], meta["slots"]
    F2, N, npad = meta["F2"], meta["N"], meta["npad"]
    assign = meta["assign"]
    out = np.empty((npad, F2), shards[0].dtype)
    for j in range(slots):
        for c in range(n_cores):
            b = assign[j, c]
            out[b * P:(b + 1) * P] = shards[c][j * P:(j + 1) * P]
    return out[:N]


# ----------------------------------------------------------------------------
# Device program
# ----------------------------------------------------------------------------

class GatherStream:
    """Issues batched dma_gathers for one (table-half, layer) idx stream and
    hands out per-chunk rhs APs."""

    def __init__(self, nc, pool, table_ap, idx_tile, total_idx, feat, tag,
                 slab_chunks=32, bufs=2):
        self.nc = nc
        self.pool = pool
        self.table_ap = table_ap
        self.idx_tile = idx_tile
        self.total = total_idx
        self.feat = feat
        self.tag = tag
        self.slab = slab_chunks
        self.bufs = bufs
        self.pos = 0              # chunk cursor
        self.cur_tile = None

    def next_chunk(self):
        s, c = divmod(self.pos, self.slab)
        if c == 0:
            base = s * self.slab * P
            n_idx = min(self.slab * P, self.total - base)
            k = n_idx // P
            t = self.pool.tile([P, self.slab, self.feat], BF16, tag=self.tag,
                               bufs=self.bufs)
            self.nc.gpsimd.dma_gather(
                out_ap=t[:, :k, :],
                in_ap=self.table_ap,
                idxs_ap=self.idx_tile[:, base // 16:(base + n_idx) // 16],
                num_idxs=n_idx,
                num_idxs_reg=n_idx,
                elem_size=self.feat,
                single_packet=False,
            )
            self.cur_tile = t
        self.pos += 1
        return self.cur_tile[:, c, :]


def build_nc(meta, slab_chunks=32, n_cores=None, collective=True, io_only=False,
             oh_batch=8, dma_scratch=65536):
    n_cores = n_cores or meta["n_cores"]
    slots, npad, half = meta["slots"], meta["npad"], meta["half"]
    F1, F2 = meta["F1"], meta["F2"]
    K1, K2 = meta["K1"], meta["K2"]
    nch1, nch2 = meta["nch1"], meta["nch2"]
    nsh = slots * P

    nc = bacc.Bacc(num_devices=n_cores, dynamic_dma_scratch_size=dma_scratch)
    dp = nc.declare_dram_parameter
    xs = dp("xs", [npad, F1], BF16, isOutput=False)
    w1 = dp("w1", [F1, F1], BF16, isOutput=False)
    w2 = dp("w2", [F1, F2], BF16, isOutput=False)
    b1r = dp("b1r", [P, F1], F32, isOutput=False)
    b2r = dp("b2r", [P, F2], F32, isOutput=False)
    iota = dp("iota", [P, 16 * P], BF16, isOutput=False)
    ident = dp("ident", [P, P], BF16, isOutput=False)
    dtgt = dp("dtgt", [P, slots], F32, isOutput=False)
    cs1 = dp("cs1", [P, nch1], BF16, isOutput=False)
    cs2 = dp("cs2", [P, nch2], BF16, isOutput=False)
    idx1l = dp("idx1l", [P, meta["tot1"][0] // 16], I16, isOutput=False)
    idx1h = dp("idx1h", [P, meta["tot1"][1] // 16], I16, isOutput=False)
    idx2l = dp("idx2l", [P, meta["tot2"][0] // 16], I16, isOutput=False)
    idx2h = dp("idx2h", [P, meta["tot2"][1] // 16], I16, isOutput=False)
    tick = dp("tick", [1, 4], F32, isOutput=False)
    out = dp("out", [nsh, F2], F32, isOutput=True)
    tock = dp("tock", [1, 4], F32, isOutput=True)

    qsize = meta.get("qsize", [slots])
    qslot0 = meta.get("qslot0", [0])
    NQ = len(qsize)
    xws2q = [nc.dram_tensor(f"xws2q{q}", [qsize[q] * P, F2], BF16)
             for q in range(NQ)]
    tab2 = nc.dram_tensor("tab2", [npad, F2], BF16, addr_space="Shared")
    q_of_slot = []
    for q in range(NQ):
        q_of_slot += [q] * qsize[q]

    AL = mybir.AluOpType
    ACT = mybir.ActivationFunctionType

    with tile.TileContext(nc) as tc:
        # NOTE: Bacc.compile() auto-inserts the GPSIMD library load for
        # dma_gather (insert_library_loads pass) -- no manual load_library.
        with (
            tc.tile_pool(name="const", bufs=1) as cpool,
            tc.tile_pool(name="msg", bufs=2) as mpool,
            tc.tile_pool(name="work", bufs=2) as wpool,
            tc.tile_pool(name="psum", bufs=2, space="PSUM") as ppool,
        ):
            # timing passthrough: tock = tick (chained-repeat measurement)
            tick_t = cpool.tile([1, 4], F32, tag="tick", bufs=1)
            nc.sync.dma_start(tick_t[:], tick[:, :])
            nc.sync.dma_start(tock[:, :], tick_t[:])

            def load_const(ap, shape, dtype, name):
                t = cpool.tile(shape, dtype, tag=name, bufs=1)
                nc.sync.dma_start(t[:], ap)
                return t

            w1_t = cpool.tile([P, 2, F1], BF16, tag="w1", bufs=1)
            for k in range(2):
                nc.sync.dma_start(w1_t[:, k, :], w1[k * P:(k + 1) * P, :])
            w2_t = cpool.tile([P, 2, F2], BF16, tag="w2", bufs=1)
            for k in range(2):
                nc.sync.dma_start(w2_t[:, k, :], w2[k * P:(k + 1) * P, :])
            b1_t = load_const(b1r[:, :], [P, F1], F32, "b1")
            b2_t = load_const(b2r[:, :], [P, F2], F32, "b2")
            io_t = load_const(iota[:, :], [P, 16 * P], BF16, "iota")
            id_t = load_const(ident[:, :], [P, P], BF16, "ident")
            dt_t = load_const(dtgt[:, :], [P, slots], F32, "dtgt")
            cs1_t = load_const(cs1[:, :], [P, nch1], BF16, "cs1")
            cs2_t = load_const(cs2[:, :], [P, nch2], BF16, "cs2")
            i1l_t = load_const(idx1l[:, :], [P, meta["tot1"][0] // 16], I16, "i1l")
            i1h_t = load_const(idx1h[:, :], [P, meta["tot1"][1] // 16], I16, "i1h")
            i2l_t = load_const(idx2l[:, :], [P, meta["tot2"][0] // 16], I16, "i2l")
            i2h_t = load_const(idx2h[:, :], [P, meta["tot2"][1] // 16], I16, "i2h")

            if io_only:
                zt = wpool.tile([P, F2], F32, tag="ep2")
                nc.vector.memset(zt[:], 0.0)
                for j in range(slots):
                    nc.sync.dma_start(out[j * P:(j + 1) * P, :], zt[:])

            hT = cpool.tile([P, 2, nsh], BF16, tag="hT", bufs=1)

            st1 = [
                GatherStream(nc, mpool, xs[0:half, :], i1l_t, meta["tot1"][0], F1, "m1l",
                             slab_chunks),
                GatherStream(nc, mpool, xs[half:npad, :], i1h_t, meta["tot1"][1], F1, "m1h",
                             slab_chunks),
            ]
            st2 = [
                GatherStream(nc, mpool, tab2[0:half, :], i2l_t, meta["tot2"][0], F2, "m2l",
                             slab_chunks),
                GatherStream(nc, mpool, tab2[half:npad, :], i2h_t, meta["tot2"][1], F2, "m2h",
                             slab_chunks),
            ]

            def aggregate(j, K, streams, cs_t, ck0, feat):
                """One-hot matmul accumulation for slot j; returns psum tile."""
                psum = ppool.tile([P, feat], F32, tag="agg")
                nch = int(K[j, 0] + K[j, 1])
                ci = 0
                ck = ck0
                for h in range(2):
                    left = int(K[j, h])
                    while left > 0:
                        nb = min(oh_batch, left)
                        # build nb one-hots in a single DVE op (iota is a
                        # real [P, nb*P] tile so the op keeps a clean layout)
                        oh = wpool.tile([P, oh_batch, P], BF16, tag="oh", bufs=3)
                        nc.vector.tensor_tensor(
                            out=oh[:, :nb, :],
                            in0=cs_t[:, ck:ck + nb, None].to_broadcast([P, nb, P]),
                            in1=io_t[:, :nb * P],
                            op=AL.is_equal,
                        )
                        for i in range(nb):
                            msg = streams[h].next_chunk()
                            nc.tensor.matmul(psum[:], lhsT=oh[:, i, :], rhs=msg,
                                             start=(ci == 0), stop=(ci == nch - 1))
                            ci += 1
                        ck += nb
                        left -= nb
                return psum, ck

            ck1 = 0
            for j in range(slots if not io_only else 0):
                # ---- layer-1 aggregation over raw x_s ----
                psum, ck1 = aggregate(j, K1, st1, cs1_t, ck1, F1)
                aggc = wpool.tile([P, F1], BF16, tag="aggc")
                nc.scalar.activation(aggc[:], psum[:], ACT.Copy)
                aggT = wpool.tile([P, 2, P], BF16, tag="aggT")
                for k in range(2):
                    ptr = ppool.tile([P, P], BF16, tag="tr")
                    nc.tensor.transpose(ptr[:], aggc[:, k * P:(k + 1) * P], id_t[:])
                    nc.vector.tensor_copy(aggT[:, k, :], ptr[:])
                # ---- dense W1 + epilogue ----
                pd = ppool.tile([P, F1], F32, tag="dense")
                for k in range(2):
                    nc.tensor.matmul(pd[:], lhsT=aggT[:, k, :], rhs=w1_t[:, k, :],
                                     start=(k == 0), stop=(k == 1))
                htile = wpool.tile([P, F1], BF16, tag="h")
                if meta.get("b1_zero", False):
                    # h = relu(dinv * (agg@W1)) in one ACT pass from PSUM
                    nc.scalar.activation(htile[:], pd[:], ACT.Relu,
                                         scale=dt_t[:, j:j + 1])
                else:
                    t1 = wpool.tile([P, F1], F32, tag="ep1")
                    nc.vector.tensor_scalar(t1[:], pd[:], dt_t[:, j:j + 1], None,
                                            op0=AL.mult)
                    nc.vector.tensor_tensor(t1[:], t1[:], b1_t[:], op=AL.add)
                    nc.scalar.activation(htile[:], t1[:], ACT.Relu)
                for k in range(2):
                    ptr = ppool.tile([P, P], BF16, tag="tr")
                    nc.tensor.transpose(ptr[:], htile[:, k * P:(k + 1) * P], id_t[:])
                    nc.vector.tensor_copy(hT[:, k, j * P:(j + 1) * P], ptr[:])
                # ---- dense W2 -> xws2 ----
                pd2 = ppool.tile([P, F2], F32, tag="dense")
                for k in range(2):
                    nc.tensor.matmul(pd2[:], lhsT=hT[:, k, j * P:(j + 1) * P],
                                     rhs=w2_t[:, k, :], start=(k == 0), stop=(k == 1))
                xw2t = wpool.tile([P, F2], BF16, tag="xw2")
                nc.scalar.activation(xw2t[:], pd2[:], ACT.Copy,
                                     scale=dt_t[:, j:j + 1])
                q = q_of_slot[j]
                jq = j - qslot0[q]
                nc.sync.dma_start(xws2q[q][jq * P:(jq + 1) * P, :], xw2t[:])
                if jq == qsize[q] - 1:
                    # last slot of this quarter: fire its sub-AllGather so it
                    # overlaps with the remaining layer-1 slots
                    r0 = qslot0[q] * n_cores * P
                    r1 = r0 + qsize[q] * n_cores * P
                    if collective:
                        nc.gpsimd.collective_compute(
                            "AllGather",
                            AL.bypass,
                            replica_groups=[list(range(n_cores))],
                            ins=[xws2q[q].ap().opt()],
                            outs=[tab2[r0:r1, :].opt()],
                        )
                    else:
                        nc.sync.dma_start(
                            tab2[r0:r0 + qsize[q] * P, :], xws2q[q][:, :])

            # ---- layer-2 aggregation + epilogue ----
            ck2 = 0
            for j in range(slots if not io_only else 0):
                psum, ck2 = aggregate(j, K2, st2, cs2_t, ck2, F2)
                t2 = wpool.tile([P, F2], F32, tag="ep2")
                if meta.get("b2_zero", False):
                    nc.scalar.activation(t2[:], psum[:], ACT.Copy,
                                         scale=dt_t[:, j:j + 1])
                else:
                    nc.vector.tensor_scalar(t2[:], psum[:], dt_t[:, j:j + 1], None,
                                            op0=AL.mult)
                    nc.vector.tensor_tensor(t2[:], t2[:], b2_t[:], op=AL.add)
                nc.sync.dma_start(out[j * P:(j + 1) * P, :], t2[:])

    nc.compile()
    return nc


class SpmdRunner:
    def __init__(self, nc, n_cores: int = 8, nreps: int = 1,
                 tick_name: str = "tick", tock_name: str = "tock"):
        install_neuronx_cc_hook()
        self.nc = nc
        self.n_cores = n_cores
        assert nc.dbg_addr is None or not nc.dbg_callbacks
        self.dbg_name = nc.dbg_addr.name if nc.dbg_addr is not None else None
        partition_name = nc.partition_id_tensor.name if nc.partition_id_tensor else None

        in_names, out_names, out_avals = [], [], []
        zero_outs = []
        for alloc in nc.m.functions[0].allocations:
            if not isinstance(alloc, mybir.MemoryLocationSet):
                continue
            name = alloc.memorylocations[0].name
            if alloc.kind == "ExternalInput":
                if name != partition_name:
                    in_names.append(name)
            elif alloc.kind == "ExternalOutput":
                out_names.append(name)
                shape = tuple(alloc.tensor_shape)
                dtype = mybir.dt.np(alloc.dtype)
                out_avals.append(jax.core.ShapedArray(shape, dtype))
                zero_outs.append(np.zeros(shape, dtype))
        self.in_names = in_names      # order matters; includes dbg if declared
        self.out_names = out_names
        self.out_avals = out_avals
        self.zero_outs = zero_outs
        n_params = len(in_names)
        n_outs = len(out_avals)
        all_in_names = list(in_names) + list(out_names)
        if partition_name is not None:
            all_in_names.append(partition_name)

        tick_i = in_names.index(tick_name) if (nreps > 1 and tick_name in in_names) else None
        tock_i = out_names.index(tock_name) if (nreps > 1 and tock_name in out_names) else None
        assert nreps == 1 or (tick_i is not None and tock_i is not None), \
            "nreps>1 needs tick/tock passthrough tensors in the kernel"

        def _call(operands):
            if partition_name is not None:
                operands = operands + [partition_id_tensor()]
            return _bass_exec_p.bind(
                *operands,
                out_avals=tuple(out_avals),
                in_names=tuple(all_in_names),
                out_names=tuple(out_names),
                lowering_input_output_aliases=(),
                sim_require_finite=True,
                sim_require_nnan=True,
                nc=nc,
            )

        def _body(*args):
            operands = list(args)
            outs = _call(list(operands))
            for _ in range(nreps - 1):
                operands2 = list(operands)
                operands2[tick_i] = outs[tock_i]
                outs = _call(operands2)
            return tuple(outs)

        devices = jax.devices()[: self.n_cores]
        assert len(devices) == self.n_cores
        mesh = Mesh(np.asarray(devices), ("core",))
        in_specs = (PartitionSpec("core"),) * (n_params + n_outs)
        out_specs = (PartitionSpec("core"),) * n_outs
        # NOTE: no donation so we can reuse the zero buffers across timed calls.
        self._fn = jax.jit(
            shard_map(_body, mesh=mesh, in_specs=in_specs, out_specs=out_specs,
                      check_rep=False),
            keep_unused=True,
        )
        self._concat_zeros = [
            np.zeros((self.n_cores * z.shape[0], *z.shape[1:]), z.dtype)
            for z in zero_outs
        ]
        self._dev_zeros = None
        self._dev_in = None

    def stage_inputs(self, in_maps):
        """in_maps: list (len n_cores) of dict name->np.ndarray."""
        if self.dbg_name is not None:
            in_maps = [
                {**m, self.dbg_name: np.zeros((1, 2), np.uint32)} for m in in_maps
            ]
        concat_in = [
            np.concatenate([np.asarray(in_maps[c][name]) for c in range(self.n_cores)],
                           axis=0)
            for name in self.in_names
        ]
        self._dev_in = [jax.device_put(a) for a in concat_in]
        self._dev_zeros = [jax.device_put(a) for a in self._concat_zeros]
        jax.block_until_ready(self._dev_in)
        jax.block_until_ready(self._dev_zeros)

    def run(self):
        outs = self._fn(*self._dev_in, *self._dev_zeros)
        jax.block_until_ready(outs)
        return outs

    def run_chain(self, n):
        """Dispatch n executions back-to-back (tick chained through tock to
        force strict ordering), block once at the end."""
        ti = self.in_names.index("tick")
        oi = self.out_names.index("tock")
        ins = list(self._dev_in)
        outs = self._fn(*ins, *self._dev_zeros)
        for _ in range(n - 1):
            ins[ti] = outs[oi]
            outs = self._fn(*ins, *self._dev_zeros)
        jax.block_until_ready(outs)
        return outs

    def results(self, outs):
        return [
            {
                name: np.asarray(outs[i]).reshape(self.n_cores, *self.out_avals[i].shape)[c]
                for i, name in enumerate(self.out_names)
            }
            for c in range(self.n_cores)
        ]


# ----------------------------------------------------------------------------
# Public entry point
# ----------------------------------------------------------------------------

_CACHE = {}


def kernel(**inputs) -> np.ndarray:
    x = np.asarray(inputs["x"], np.float32)
    edge_index = np.asarray(inputs["edge_index"], np.int64)
    W1 = np.asarray(inputs["W1"], np.float32)
    b1 = np.asarray(inputs["b1"], np.float32)
    W2 = np.asarray(inputs["W2"], np.float32)
    b2 = np.asarray(inputs["b2"], np.float32)

    in_maps, meta = plan_host(x, edge_index, W1, b1, W2, b2)
    for m in in_maps:
        m["tick"] = np.zeros((1, 4), np.float32)

    key = (x.shape, edge_index.shape, W2.shape,
           tuple(meta["K1"].reshape(-1)), tuple(meta["K2"].reshape(-1)),
           meta["b1_zero"], meta["b2_zero"])
    if key not in _CACHE:
        nc = build_nc(meta, slab_chunks=24, oh_batch=16)
        _CACHE[key] = SpmdRunner(nc, meta["n_cores"])
    runner = _CACHE[key]
    runner.stage_inputs(in_maps)
    outs = runner.run()
    res = runner.results(outs)
    shards = [res[c]["out"] for c in range(meta["n_cores"])]
    return assemble_output(shards, meta).astype(np.float32)


# revision 7
# speedup vs baseline: 9.5137x; 1.5045x over previous
"""Self-contained 2-layer GCN kernel for 8 Trainium2 NeuronCores.

kernel(**inputs) takes the FULL unsharded inputs (x, edge_index, W1, b1,
W2, b2) and returns the full [N, 128] float32 output.

Design:
- Target-node blocks (128 nodes) are load-balanced across (core, slot)
  pairs so all 8 cores run one identical SPMD program; per-core work is
  equalized via a shared per-slot chunk-count template (shortfall padded
  with dummy edges whose one-hot column is -1 -> zero contribution).
- Layer 1 uses (A_hat X) W1 associativity: edges gather rows of the
  host-prepared dinv*x table (bf16) with dma_gather, a 0/1 one-hot
  [slot,target] matrix is built on VectorE (is_equal vs iota, 16 chunks
  per op) and each 128-edge chunk accumulates into PSUM on TensorE:
  psum += oh.T @ msg.
- Per-slot epilogue: transpose agg (PE), dense W1 (+relu, dinv scale on
  ScalarE), transpose h, dense W2 -> xws2 (bf16).
- xws2 is AllGathered into the layer-2 table in 5 tapered slot-group
  sub-collectives (small first group) so the exchange chain streams while
  layer-1 is still computing;
  layer-2 repeats the aggregation at F=128 and writes f32 output shards.
- dma_gather indices are int16, so tables are split in lo/hi halves of
  25088 rows; gathers run as 24-chunk (3072-index) slabs with
  single_packet=False (larger slabs hang the SWDGE path otherwise).
"""
import numpy as np
import ml_dtypes

import jax
from jax.sharding import Mesh, PartitionSpec
from jax.experimental.shard_map import shard_map

import concourse.bacc as bacc
import concourse.mybir as mybir
import concourse.tile as tile
from concourse.bass2jax import _bass_exec_p, install_neuronx_cc_hook, partition_id_tensor

P = 128
F32 = mybir.dt.float32
BF16 = mybir.dt.bfloat16
I16 = mybir.dt.int16
NP_BF16 = ml_dtypes.bfloat16


# ----------------------------------------------------------------------------
# Host-side planning
# ----------------------------------------------------------------------------

def _pack_idx(vals: np.ndarray) -> np.ndarray:
    """Pack an int16 index stream into the [128, n/16] dma_gather layout.

    Position i is read from idxs[i % 16, i // 16]; the 16-row pattern is
    replicated 8x down the partitions (one copy per Q7 core).
    """
    n = len(vals)
    assert n % 16 == 0
    arr16 = np.asarray(vals, np.int16).reshape(n // 16, 16).T  # [16, n/16]
    return np.tile(arr16, (8, 1))  # [128, n/16]


class LayerPlan:
    """Per-layer gather/one-hot plan: per-core idx streams + csel + template."""

    def __init__(self, srcs, tgt_core, tgt_slot, tgt_off, n_cores, slots, half):
        # group edges by (core, slot, half-of-source)
        e_half = (srcs >= half).astype(np.int64)
        e_idx = np.where(e_half == 0, srcs, srcs - half).astype(np.int64)
        assert e_idx.max() < 2 ** 15
        key = ((tgt_core * slots + tgt_slot) * 2 + e_half)
        order = np.argsort(key, kind="stable")
        key_s = key[order]
        idx_s = e_idx[order]
        off_s = tgt_off[order]
        n_groups = n_cores * slots * 2
        counts = np.bincount(key_s, minlength=n_groups).reshape(n_cores, slots, 2)
        chunks = -(-counts // P)  # ceil div
        # template: per (slot, half) chunk count = max over cores
        self.K = chunks.max(axis=0)  # [slots, 2]
        starts = np.concatenate([[0], np.cumsum(counts.reshape(-1))])
        self.idx_streams = []   # per core: (lo_vals, hi_vals)
        self.csel = []          # per core: [128, n_chunks] float (-1 pad)
        nch = int(self.K.sum())
        self.n_chunks = nch
        for c in range(n_cores):
            lo_parts, hi_parts = [], []
            cs = np.full((nch, P), -1.0, np.float32)
            ck = 0
            for j in range(slots):
                for h in range(2):
                    g = (c * slots + j) * 2 + h
                    cnt = counts[c, j, h]
                    kk = int(self.K[j, h])
                    vals = np.zeros(kk * P, np.int64)
                    sel = np.full(kk * P, -1.0, np.float32)
                    vals[:cnt] = idx_s[starts[g]:starts[g] + cnt]
                    sel[:cnt] = off_s[starts[g]:starts[g] + cnt]
                    (lo_parts if h == 0 else hi_parts).append(vals)
                    cs[ck:ck + kk] = sel.reshape(kk, P)
                    ck += kk
            lo = np.concatenate(lo_parts) if lo_parts else np.zeros(0, np.int64)
            hi = np.concatenate(hi_parts) if hi_parts else np.zeros(0, np.int64)
            self.idx_streams.append((lo, hi))
            self.csel.append(cs.T.copy())  # [128, n_chunks]
        self.tot = (int(self.K[:, 0].sum()) * P, int(self.K[:, 1].sum()) * P)


def plan_host(x, edge_index, W1, b1, W2, b2, n_cores=8):
    N, F1 = x.shape
    F2 = W2.shape[1]
    row = np.asarray(edge_index[0], np.int64)
    col = np.asarray(edge_index[1], np.int64)

    nb = -(-N // P)
    nbp = -(-nb // n_cores) * n_cores          # padded #blocks (392)
    slots = nbp // n_cores                     # 49
    npad = nbp * P                             # 50176
    half = npad // 2                           # 25088
    assert half % P == 0 and half < 2 ** 15

    deg = np.bincount(col, minlength=N).astype(np.float64) + 1.0
    dinv = (deg ** -0.5).astype(np.float32)

    # all edges incl. self loops
    loops = np.arange(N, dtype=np.int64)
    srcs = np.concatenate([row, loops])
    tgts = np.concatenate([col, loops])

    # --- balance target blocks across (core, slot) ---
    # The per-(slot, half) chunk template is max over the 8 blocks in the
    # band, so band blocks with matching (lo, hi) chunk counts minimize
    # padding. Lexsort by (ceil(lo/P), ceil(hi/P)) then greedy-swap refine.
    blk = tgts // P
    lo_cnt = np.bincount(blk[srcs < half], minlength=nbp)
    hi_cnt = np.bincount(blk[srcs >= half], minlength=nbp)
    lo_ch = np.ceil(lo_cnt / P).astype(np.int64)
    hi_ch = np.ceil(hi_cnt / P).astype(np.int64)
    order = np.lexsort((-hi_ch, -lo_ch))
    assign = order.reshape(slots, n_cores).copy()  # assign[j, c] = block id

    def band_cost(band):
        return lo_ch[band].max() + hi_ch[band].max()

    costs = np.array([band_cost(assign[j]) for j in range(slots)])
    rng_ = np.random.default_rng(0)
    for _ in range(20000):
        j1, j2 = rng_.integers(0, slots, 2)
        if j1 == j2:
            continue
        c1, c2 = rng_.integers(0, n_cores, 2)
        b1_, b2_ = assign[j1, c1], assign[j2, c2]
        assign[j1, c1], assign[j2, c2] = b2_, b1_
        n1, n2 = band_cost(assign[j1]), band_cost(assign[j2])
        if n1 + n2 < costs[j1] + costs[j2]:
            costs[j1], costs[j2] = n1, n2
        else:
            assign[j1, c1], assign[j2, c2] = b1_, b2_
    # order bands small-first: smooths the tail and fires the first
    # sub-AllGather earlier in wall-clock
    band_tot = np.array([band_cost(assign[j]) for j in range(slots)])
    assign = assign[np.argsort(band_tot, kind="stable")]
    core_of_blk = np.empty(nbp, np.int64)
    slot_of_blk = np.empty(nbp, np.int64)
    new_base = np.empty(nbp, np.int64)
    # layer-2 table layout is quarter-major: [(quarter, core, slot-in-q), 128]
    # so the AllGather can run as 4 contiguous sub-collectives overlapped
    # with layer-1 compute.
    # tapered split: fire the first sub-collective early so the AG chain
    # streams while layer-1 is still computing; later quarters grow.
    if slots >= 16:
        frac = np.array([3, 9, 9, 12, 16], np.float64)
        qsize = np.maximum(1, np.floor(frac / frac.sum() * slots)).astype(int)
        qsize[-1] += slots - qsize.sum()
        qsize = [int(v) for v in qsize]
    else:
        qsize = [slots // 4 + (1 if q < slots % 4 else 0) for q in range(4)]
        qsize = [v for v in qsize if v > 0]
    NQ = len(qsize)
    qslot0 = np.concatenate([[0], np.cumsum(qsize)])[:NQ]
    quarter_of_slot = np.repeat(np.arange(NQ), qsize)
    for j in range(slots):
        q = quarter_of_slot[j]
        for c in range(n_cores):
            b = assign[j, c]
            core_of_blk[b] = c
            slot_of_blk[b] = j
            new_base[b] = (int(qslot0[q]) * n_cores + c * qsize[q]
                           + (j - int(qslot0[q]))) * P
    new_row = new_base[np.arange(npad) // P] + np.arange(npad) % P  # node -> table2 row

    tgt_core = core_of_blk[blk]
    tgt_slot = slot_of_blk[blk]
    tgt_off = (tgts % P).astype(np.float32)

    l1 = LayerPlan(srcs, tgt_core, tgt_slot, tgt_off, n_cores, slots, half)
    l2 = LayerPlan(new_row[srcs], tgt_core, tgt_slot, tgt_off, n_cores, slots, half)

    # --- tables / constants ---
    xs = np.zeros((npad, F1), NP_BF16)
    xs[:N] = (x.astype(np.float32) * dinv[:, None]).astype(NP_BF16)

    dinv_pad = np.zeros(npad, np.float32)
    dinv_pad[:N] = dinv
    iota = np.tile(np.arange(P, dtype=np.float32), (P, 16)).astype(NP_BF16)
    ident = np.eye(P, dtype=np.float32).astype(NP_BF16)

    in_maps = []
    for c in range(n_cores):
        m = {
            "xs": xs,
            "w1": W1.astype(np.float32).astype(NP_BF16),
            "w2": W2.astype(np.float32).astype(NP_BF16),
            "b1r": np.tile(np.asarray(b1, np.float32), (P, 1)),
            "b2r": np.tile(np.asarray(b2, np.float32), (P, 1)),
            "iota": iota,
            "ident": ident,
            # dinv of this core's blocks, [128, slots] (partition = within-block)
            "dtgt": dinv_pad[assign[:, c][:, None] * P
                             + np.arange(P)[None, :]].T.copy(),
            "cs1": l1.csel[c].astype(NP_BF16),
            "cs2": l2.csel[c].astype(NP_BF16),
            "idx1l": _pack_idx(l1.idx_streams[c][0]),
            "idx1h": _pack_idx(l1.idx_streams[c][1]),
            "idx2l": _pack_idx(l2.idx_streams[c][0]),
            "idx2h": _pack_idx(l2.idx_streams[c][1]),
        }
        in_maps.append(m)

    meta = {
        "N": N, "F1": F1, "F2": F2, "n_cores": n_cores,
        "b1_zero": bool(np.all(np.asarray(b1) == 0)),
        "b2_zero": bool(np.all(np.asarray(b2) == 0)),
        "slots": slots, "npad": npad, "half": half,
        "K1": l1.K, "K2": l2.K,
        "tot1": l1.tot, "tot2": l2.tot,
        "nch1": l1.n_chunks, "nch2": l2.n_chunks,
        "assign": assign,
        "qsize": qsize, "qslot0": [int(v) for v in qslot0],
    }
    return in_maps, meta


def assemble_output(shards, meta):
    """shards: list per core of [slots*128, F2] -> full [N, F2]."""
    n_cores, slots = meta["n_cores"], meta["slots"]
    F2, N, npad = meta["F2"], meta["N"], meta["npad"]
    assign = meta["assign"]
    out = np.empty((npad, F2), shards[0].dtype)
    for j in range(slots):
        for c in range(n_cores):
            b = assign[j, c]
            out[b * P:(b + 1) * P] = shards[c][j * P:(j + 1) * P]
    return out[:N]


# ----------------------------------------------------------------------------
# Device program
# ----------------------------------------------------------------------------

class GatherStream:
    """Issues batched dma_gathers for one (table-half, layer) idx stream and
    hands out per-chunk rhs APs."""

    def __init__(self, nc, pool, table_ap, idx_tile, total_idx, feat, tag,
                 slab_chunks=32, bufs=2):
        self.nc = nc
        self.pool = pool
        self.table_ap = table_ap
        self.idx_tile = idx_tile
        self.total = total_idx
        self.feat = feat
        self.tag = tag
        self.slab = slab_chunks
        self.bufs = bufs
        self.pos = 0              # chunk cursor
        self.cur_tile = None

    def next_chunk(self):
        s, c = divmod(self.pos, self.slab)
        if c == 0:
            base = s * self.slab * P
            n_idx = min(self.slab * P, self.total - base)
            k = n_idx // P
            t = self.pool.tile([P, self.slab, self.feat], BF16, tag=self.tag,
                               bufs=self.bufs)
            self.nc.gpsimd.dma_gather(
                out_ap=t[:, :k, :],
                in_ap=self.table_ap,
                idxs_ap=self.idx_tile[:, base // 16:(base + n_idx) // 16],
                num_idxs=n_idx,
                num_idxs_reg=n_idx,
                elem_size=self.feat,
                single_packet=False,
            )
            self.cur_tile = t
        self.pos += 1
        return self.cur_tile[:, c, :]


def build_nc(meta, slab_chunks=32, n_cores=None, collective=True, io_only=False,
             oh_batch=8, dma_scratch=65536):
    n_cores = n_cores or meta["n_cores"]
    slots, npad, half = meta["slots"], meta["npad"], meta["half"]
    F1, F2 = meta["F1"], meta["F2"]
    K1, K2 = meta["K1"], meta["K2"]
    nch1, nch2 = meta["nch1"], meta["nch2"]
    nsh = slots * P

    nc = bacc.Bacc(num_devices=n_cores, dynamic_dma_scratch_size=dma_scratch)
    dp = nc.declare_dram_parameter
    xs = dp("xs", [npad, F1], BF16, isOutput=False)
    w1 = dp("w1", [F1, F1], BF16, isOutput=False)
    w2 = dp("w2", [F1, F2], BF16, isOutput=False)
    b1r = dp("b1r", [P, F1], F32, isOutput=False)
    b2r = dp("b2r", [P, F2], F32, isOutput=False)
    iota = dp("iota", [P, 16 * P], BF16, isOutput=False)
    ident = dp("ident", [P, P], BF16, isOutput=False)
    dtgt = dp("dtgt", [P, slots], F32, isOutput=False)
    cs1 = dp("cs1", [P, nch1], BF16, isOutput=False)
    cs2 = dp("cs2", [P, nch2], BF16, isOutput=False)
    idx1l = dp("idx1l", [P, meta["tot1"][0] // 16], I16, isOutput=False)
    idx1h = dp("idx1h", [P, meta["tot1"][1] // 16], I16, isOutput=False)
    idx2l = dp("idx2l", [P, meta["tot2"][0] // 16], I16, isOutput=False)
    idx2h = dp("idx2h", [P, meta["tot2"][1] // 16], I16, isOutput=False)
    tick = dp("tick", [1, 4], F32, isOutput=False)
    out = dp("out", [nsh, F2], F32, isOutput=True)
    tock = dp("tock", [1, 4], F32, isOutput=True)

    qsize = meta.get("qsize", [slots])
    qslot0 = meta.get("qslot0", [0])
    NQ = len(qsize)
    xws2q = [nc.dram_tensor(f"xws2q{q}", [qsize[q] * P, F2], BF16)
             for q in range(NQ)]
    tab2 = nc.dram_tensor("tab2", [npad, F2], BF16, addr_space="Shared")
    q_of_slot = []
    for q in range(NQ):
        q_of_slot += [q] * qsize[q]

    AL = mybir.AluOpType
    ACT = mybir.ActivationFunctionType

    with tile.TileContext(nc) as tc:
        # NOTE: Bacc.compile() auto-inserts the GPSIMD library load for
        # dma_gather (insert_library_loads pass) -- no manual load_library.
        with (
            tc.tile_pool(name="const", bufs=1) as cpool,
            tc.tile_pool(name="msg", bufs=2) as mpool,
            tc.tile_pool(name="work", bufs=2) as wpool,
            tc.tile_pool(name="psum", bufs=2, space="PSUM") as ppool,
        ):
            # timing passthrough: tock = tick (chained-repeat measurement)
            tick_t = cpool.tile([1, 4], F32, tag="tick", bufs=1)
            nc.sync.dma_start(tick_t[:], tick[:, :])
            nc.sync.dma_start(tock[:, :], tick_t[:])

            def load_const(ap, shape, dtype, name):
                t = cpool.tile(shape, dtype, tag=name, bufs=1)
                nc.sync.dma_start(t[:], ap)
                return t

            w1_t = cpool.tile([P, 2, F1], BF16, tag="w1", bufs=1)
            for k in range(2):
                nc.sync.dma_start(w1_t[:, k, :], w1[k * P:(k + 1) * P, :])
            w2_t = cpool.tile([P, 2, F2], BF16, tag="w2", bufs=1)
            for k in range(2):
                nc.sync.dma_start(w2_t[:, k, :], w2[k * P:(k + 1) * P, :])
            b1_t = load_const(b1r[:, :], [P, F1], F32, "b1")
            b2_t = load_const(b2r[:, :], [P, F2], F32, "b2")
            io_t = load_const(iota[:, :], [P, 16 * P], BF16, "iota")
            id_t = load_const(ident[:, :], [P, P], BF16, "ident")
            dt_t = load_const(dtgt[:, :], [P, slots], F32, "dtgt")
            cs1_t = load_const(cs1[:, :], [P, nch1], BF16, "cs1")
            cs2_t = load_const(cs2[:, :], [P, nch2], BF16, "cs2")
            # layer-1 and layer-2 idx tiles share one pool slot each (their
            # lifetimes are disjoint: L2 gathers start after the last L1 one)
            i1l_t = load_const(idx1l[:, :], [P, meta["tot1"][0] // 16], I16, "ixl")
            i1h_t = load_const(idx1h[:, :], [P, meta["tot1"][1] // 16], I16, "ixh")
            i2l_t = cpool.tile([P, meta["tot2"][0] // 16], I16, tag="ixl", bufs=1)
            i2h_t = cpool.tile([P, meta["tot2"][1] // 16], I16, tag="ixh", bufs=1)
            nc.sync.dma_start(i2l_t[:], idx2l[:, :])
            nc.sync.dma_start(i2h_t[:], idx2h[:, :])

            if io_only:
                zt = wpool.tile([P, F2], F32, tag="ep2")
                nc.vector.memset(zt[:], 0.0)
                for j in range(slots):
                    nc.sync.dma_start(out[j * P:(j + 1) * P, :], zt[:])

            hT = cpool.tile([P, 2, nsh], BF16, tag="hT", bufs=1)

            st1 = [
                GatherStream(nc, mpool, xs[0:half, :], i1l_t, meta["tot1"][0], F1, "m1l",
                             slab_chunks),
                GatherStream(nc, mpool, xs[half:npad, :], i1h_t, meta["tot1"][1], F1, "m1h",
                             slab_chunks),
            ]
            st2 = [
                GatherStream(nc, mpool, tab2[0:half, :], i2l_t, meta["tot2"][0], F2, "m2l",
                             slab_chunks),
                GatherStream(nc, mpool, tab2[half:npad, :], i2h_t, meta["tot2"][1], F2, "m2h",
                             slab_chunks),
            ]

            def aggregate(j, K, streams, cs_t, ck0, feat):
                """One-hot matmul accumulation for slot j; returns psum tile."""
                psum = ppool.tile([P, feat], F32, tag="agg")
                nch = int(K[j, 0] + K[j, 1])
                ci = 0
                ck = ck0
                for h in range(2):
                    left = int(K[j, h])
                    while left > 0:
                        nb = min(oh_batch, left)
                        # build nb one-hots in a single DVE op (iota is a
                        # real [P, nb*P] tile so the op keeps a clean layout)
                        oh = wpool.tile([P, oh_batch, P], BF16, tag="oh", bufs=3)
                        nc.vector.tensor_tensor(
                            out=oh[:, :nb, :],
                            in0=cs_t[:, ck:ck + nb, None].to_broadcast([P, nb, P]),
                            in1=io_t[:, :nb * P],
                            op=AL.is_equal,
                        )
                        for i in range(nb):
                            msg = streams[h].next_chunk()
                            nc.tensor.matmul(psum[:], lhsT=oh[:, i, :], rhs=msg,
                                             start=(ci == 0), stop=(ci == nch - 1))
                            ci += 1
                        ck += nb
                        left -= nb
                return psum, ck

            ck1 = 0
            for j in range(slots if not io_only else 0):
                # ---- layer-1 aggregation over raw x_s ----
                psum, ck1 = aggregate(j, K1, st1, cs1_t, ck1, F1)
                aggc = wpool.tile([P, F1], BF16, tag="aggc")
                nc.scalar.activation(aggc[:], psum[:], ACT.Copy)
                aggT = wpool.tile([P, 2, P], BF16, tag="aggT")
                for k in range(2):
                    ptr = ppool.tile([P, P], BF16, tag="tr")
                    nc.tensor.transpose(ptr[:], aggc[:, k * P:(k + 1) * P], id_t[:])
                    nc.vector.tensor_copy(aggT[:, k, :], ptr[:])
                # ---- dense W1 + epilogue ----
                pd = ppool.tile([P, F1], F32, tag="dense")
                for k in range(2):
                    nc.tensor.matmul(pd[:], lhsT=aggT[:, k, :], rhs=w1_t[:, k, :],
                                     start=(k == 0), stop=(k == 1))
                htile = wpool.tile([P, F1], BF16, tag="h")
                if meta.get("b1_zero", False):
                    # h = relu(dinv * (agg@W1)) in one ACT pass from PSUM
                    nc.scalar.activation(htile[:], pd[:], ACT.Relu,
                                         scale=dt_t[:, j:j + 1])
                else:
                    t1 = wpool.tile([P, F1], F32, tag="ep1")
                    nc.vector.tensor_scalar(t1[:], pd[:], dt_t[:, j:j + 1], None,
                                            op0=AL.mult)
                    nc.vector.tensor_tensor(t1[:], t1[:], b1_t[:], op=AL.add)
                    nc.scalar.activation(htile[:], t1[:], ACT.Relu)
                for k in range(2):
                    ptr = ppool.tile([P, P], BF16, tag="tr")
                    nc.tensor.transpose(ptr[:], htile[:, k * P:(k + 1) * P], id_t[:])
                    nc.vector.tensor_copy(hT[:, k, j * P:(j + 1) * P], ptr[:])
                # ---- dense W2 -> xws2 ----
                pd2 = ppool.tile([P, F2], F32, tag="dense")
                for k in range(2):
                    nc.tensor.matmul(pd2[:], lhsT=hT[:, k, j * P:(j + 1) * P],
                                     rhs=w2_t[:, k, :], start=(k == 0), stop=(k == 1))
                xw2t = wpool.tile([P, F2], BF16, tag="xw2")
                nc.scalar.activation(xw2t[:], pd2[:], ACT.Copy,
                                     scale=dt_t[:, j:j + 1])
                q = q_of_slot[j]
                jq = j - qslot0[q]
                nc.sync.dma_start(xws2q[q][jq * P:(jq + 1) * P, :], xw2t[:])
                if jq == qsize[q] - 1:
                    # last slot of this quarter: fire its sub-AllGather so it
                    # overlaps with the remaining layer-1 slots
                    r0 = qslot0[q] * n_cores * P
                    r1 = r0 + qsize[q] * n_cores * P
                    if collective:
                        nc.gpsimd.collective_compute(
                            "AllGather",
                            AL.bypass,
                            replica_groups=[list(range(n_cores))],
                            ins=[xws2q[q].ap().opt()],
                            outs=[tab2[r0:r1, :].opt()],
                        )
                    else:
                        nc.sync.dma_start(
                            tab2[r0:r0 + qsize[q] * P, :], xws2q[q][:, :])

            # ---- layer-2 aggregation + epilogue ----
            ck2 = 0
            for j in range(slots if not io_only else 0):
                psum, ck2 = aggregate(j, K2, st2, cs2_t, ck2, F2)
                t2 = wpool.tile([P, F2], F32, tag="ep2")
                if meta.get("b2_zero", False):
                    nc.scalar.activation(t2[:], psum[:], ACT.Copy,
                                         scale=dt_t[:, j:j + 1])
                else:
                    nc.vector.tensor_scalar(t2[:], psum[:], dt_t[:, j:j + 1], None,
                                            op0=AL.mult)
                    nc.vector.tensor_tensor(t2[:], t2[:], b2_t[:], op=AL.add)
                nc.sync.dma_start(out[j * P:(j + 1) * P, :], t2[:])

    nc.compile()
    return nc

class SpmdRunner:
    def __init__(self, nc, n_cores: int = 8, nreps: int = 1,
                 tick_name: str = "tick", tock_name: str = "tock"):
        install_neuronx_cc_hook()
        self.nc = nc
        self.n_cores = n_cores
        assert nc.dbg_addr is None or not nc.dbg_callbacks
        self.dbg_name = nc.dbg_addr.name if nc.dbg_addr is not None else None
        partition_name = nc.partition_id_tensor.name if nc.partition_id_tensor else None

        in_names, out_names, out_avals = [], [], []
        zero_outs = []
        for alloc in nc.m.functions[0].allocations:
            if not isinstance(alloc, mybir.MemoryLocationSet):
                continue
            name = alloc.memorylocations[0].name
            if alloc.kind == "ExternalInput":
                if name != partition_name:
                    in_names.append(name)
            elif alloc.kind == "ExternalOutput":
                out_names.append(name)
                shape = tuple(alloc.tensor_shape)
                dtype = mybir.dt.np(alloc.dtype)
                out_avals.append(jax.core.ShapedArray(shape, dtype))
                zero_outs.append(np.zeros(shape, dtype))
        self.in_names = in_names      # order matters; includes dbg if declared
        self.out_names = out_names
        self.out_avals = out_avals
        self.zero_outs = zero_outs
        n_params = len(in_names)
        n_outs = len(out_avals)
        all_in_names = list(in_names) + list(out_names)
        if partition_name is not None:
            all_in_names.append(partition_name)

        tick_i = in_names.index(tick_name) if (nreps > 1 and tick_name in in_names) else None
        tock_i = out_names.index(tock_name) if (nreps > 1 and tock_name in out_names) else None
        assert nreps == 1 or (tick_i is not None and tock_i is not None), \
            "nreps>1 needs tick/tock passthrough tensors in the kernel"

        def _call(operands):
            if partition_name is not None:
                operands = operands + [partition_id_tensor()]
            return _bass_exec_p.bind(
                *operands,
                out_avals=tuple(out_avals),
                in_names=tuple(all_in_names),
                out_names=tuple(out_names),
                lowering_input_output_aliases=(),
                sim_require_finite=True,
                sim_require_nnan=True,
                nc=nc,
            )

        def _body(*args):
            operands = list(args)
            outs = _call(list(operands))
            for _ in range(nreps - 1):
                operands2 = list(operands)
                operands2[tick_i] = outs[tock_i]
                outs = _call(operands2)
            return tuple(outs)

        devices = jax.devices()[: self.n_cores]
        assert len(devices) == self.n_cores
        mesh = Mesh(np.asarray(devices), ("core",))
        in_specs = (PartitionSpec("core"),) * (n_params + n_outs)
        out_specs = (PartitionSpec("core"),) * n_outs
        # NOTE: no donation so we can reuse the zero buffers across timed calls.
        self._fn = jax.jit(
            shard_map(_body, mesh=mesh, in_specs=in_specs, out_specs=out_specs,
                      check_rep=False),
            keep_unused=True,
        )
        self._concat_zeros = [
            np.zeros((self.n_cores * z.shape[0], *z.shape[1:]), z.dtype)
            for z in zero_outs
        ]
        self._dev_zeros = None
        self._dev_in = None

    def stage_inputs(self, in_maps):
        """in_maps: list (len n_cores) of dict name->np.ndarray."""
        if self.dbg_name is not None:
            in_maps = [
                {**m, self.dbg_name: np.zeros((1, 2), np.uint32)} for m in in_maps
            ]
        concat_in = [
            np.concatenate([np.asarray(in_maps[c][name]) for c in range(self.n_cores)],
                           axis=0)
            for name in self.in_names
        ]
        self._dev_in = [jax.device_put(a) for a in concat_in]
        self._dev_zeros = [jax.device_put(a) for a in self._concat_zeros]
        jax.block_until_ready(self._dev_in)
        jax.block_until_ready(self._dev_zeros)

    def run(self):
        outs = self._fn(*self._dev_in, *self._dev_zeros)
        jax.block_until_ready(outs)
        return outs

    def run_chain(self, n):
        """Dispatch n executions back-to-back (tick chained through tock to
        force strict ordering), block once at the end."""
        ti = self.in_names.index("tick")
        oi = self.out_names.index("tock")
        ins = list(self._dev_in)
        outs = self._fn(*ins, *self._dev_zeros)
        for _ in range(n - 1):
            ins[ti] = outs[oi]
            outs = self._fn(*ins, *self._dev_zeros)
        jax.block_until_ready(outs)
        return outs

    def results(self, outs):
        return [
            {
                name: np.asarray(outs[i]).reshape(self.n_cores, *self.out_avals[i].shape)[c]
                for i, name in enumerate(self.out_names)
            }
            for c in range(self.n_cores)
        ]


# ----------------------------------------------------------------------------
# Public entry point
# ----------------------------------------------------------------------------

_CACHE = {}


def kernel(**inputs) -> np.ndarray:
    x = np.asarray(inputs["x"], np.float32)
    edge_index = np.asarray(inputs["edge_index"], np.int64)
    W1 = np.asarray(inputs["W1"], np.float32)
    b1 = np.asarray(inputs["b1"], np.float32)
    W2 = np.asarray(inputs["W2"], np.float32)
    b2 = np.asarray(inputs["b2"], np.float32)

    in_maps, meta = plan_host(x, edge_index, W1, b1, W2, b2)
    for m in in_maps:
        m["tick"] = np.zeros((1, 4), np.float32)

    key = (x.shape, edge_index.shape, W2.shape,
           tuple(meta["K1"].reshape(-1)), tuple(meta["K2"].reshape(-1)),
           meta["b1_zero"], meta["b2_zero"])
    if key not in _CACHE:
        nc = build_nc(meta, slab_chunks=24, oh_batch=16)
        _CACHE[key] = SpmdRunner(nc, meta["n_cores"])
    runner = _CACHE[key]
    runner.stage_inputs(in_maps)
    outs = runner.run()
    res = runner.results(outs)
    shards = [res[c]["out"] for c in range(meta["n_cores"])]
    return assemble_output(shards, meta).astype(np.float32)
